# revision 20
# baseline (speedup 1.0000x reference)
"""Trainium2 Bass kernel for BaselineFeedforwardNetwork (dense_mlp).

Computation (per path n, step t):
    x_t   = [f_t (3), delta_{t-1} (1)]
    h     = relu(x_t @ W1 + b1)        # 4  -> 64
    h2    = relu(h @ W2 + b2)          # 64 -> 64
    delta = h2 @ W3 + b3               # 64 -> 1
Output: deltas (N, T).

Strategy (8 NeuronCores, pure data parallel over N):
  * hidden dim on SBUF partitions, paths on the free axis; two chunks
    of 512 paths stacked per 128 partitions
  * per core: 32768 paths = 16 passes of 2048; 4 passes ("vlanes") run
    concurrently, each owning 2 PSUM banks (in-place pre1->pre2->delta),
    so 4 independent per-step dependency chains overlap and keep all
    engines busy across the serial T recurrence
  * delta feedback folded into layer 1 as the rank-1 matrix
    M = W3 @ W1[3,:] consuming h2_{t-1} (no PSUM->SBUF copy on the
    feedback path)
  * matmul emission is stage-major, rotating vlane x chunk-parity so
    consecutive PE instructions hit disjoint 64x64 quadrant sets and
    overlap in the array (measured 65 ns/MM for 4-way rotation vs 436
    serial); per-chunk accumulation groups (mm1a start -> mmM stop)
    are interleaved across banks
  * mm3: one K=128 matmul per chunk pair computes both parities'
    deltas via a select-column lhsT; one accumulation group per vlane
    at col-group 32*(v%2)
  * activations alternate ACT/DVE per vlane to balance the engines
"""

import sys

for _p in ("/opt/trn_rl_repo",):
    if _p not in sys.path:
        sys.path.insert(0, _p)

import os
import numpy as np
import ml_dtypes

NCORES = 8
N_TOT, T, FDIM = 262144, 60, 3
NC = N_TOT // NCORES          # 32768 paths per core
HID = 64
CH = 512                      # matmul free dim (one PSUM bank of fp32)
G = 4                         # chunks per vlane-step
GP = G * CH                   # 2048 paths per pass
NPASS = NC // GP              # 16
NLANES = 4                    # passes in lockstep (2 PSUM banks each)

# wpack column layout (all bf16, 128 partitions; W1a/M/W2 duplicated on
# both partition halves so any chunk parity / vlane can read it)
W1A_OFF = 0                                  # rows {0:3, 64:67} = W1[0:3]
M_OFF = 64                                   # rows 0:64 / 64:128 = W3 @ W1[3,:] (rank-1 fold)
W2_OFF = M_OFF + 64                          # rows 0:64 / 64:128 = W2
W3_OFF = W2_OFF + 64                         # pair-select blocks [128,32]:
                                             #   block i: W3 at (rows 0:64, col i)
                                             #   and (rows 64:128, col 16+i)
WCOLS = W3_OFF + 32                          # 224
DBANK = 1                                    # PSUM bank (col block) holding deltas


def _build_graph(npass=NPASS, nsteps=T, b3val=0.0):
    import concourse.bacc as bacc
    from concourse import mybir
    from concourse.tile import TileContext

    BF = mybir.dt.bfloat16
    F32 = mybir.dt.float32

    import time as _time

    nc = bacc.Bacc(trn_type="TRN2", name=f"k{int(_time.time())}")

    feats_p = nc.declare_dram_parameter("feats", [T, FDIM, NC], BF, isOutput=False)
    wpack_p = nc.declare_dram_parameter("wpack", [128, WCOLS], BF, isOutput=False)
    bias_p = nc.declare_dram_parameter("biasp", [128, 4], F32, isOutput=False)
    out_p = nc.declare_dram_parameter(
        "out", [T, 2, NPASS * G // 2, CH], BF, isOutput=True
    )

    NL = NLANES

    with TileContext(nc) as tc:
        with (
            tc.tile_pool(name="consts", bufs=1) as cpool,
            tc.tile_pool(name="sbuf", bufs=2) as spool,
            tc.tile_pool(name="xqp", bufs=8) as xpool,
            tc.tile_pool(name="psum", bufs=1, space="PSUM") as ppool,
        ):
            wp = cpool.tile([128, WCOLS], BF, tag="wpack")
            bp = cpool.tile([128, 4], F32, tag="biasp")
            nc.sync.dma_start(out=wp[:, :], in_=wpack_p[:, :])
            nc.sync.dma_start(out=bp[:, :], in_=bias_p[:, :])

            # Warm-up: loads the ACT table + lets ACT/DVE observe const DMAs
            warm = cpool.tile([128, 4], F32, tag="warm")
            nc.scalar.activation(
                warm[:, 0:1], bp[:, 0:1],
                mybir.ActivationFunctionType.Relu, bias=0.0, scale=1.0,
            )
            nc.vector.tensor_scalar(
                warm[:, 1:2], bp[:, 1:2], 0.0, None, mybir.AluOpType.add,
            )

            def dma_x(v, p, t0):
                # feature loads go through GpSimd SWDGE: the engine is
                # otherwise idle and these are prefetched XPRE steps
                # ahead, so the ~1-2us SWDGE latency is hidden while the
                # saturated Sync sequencer keeps only the output DMAs
                xt = xpool.tile([67, GP], BF, tag=f"xq{v}")
                nc.gpsimd.dma_start(
                    out=xt[0:FDIM, :], in_=feats_p[t0, :, p * GP : (p + 1) * GP]
                )
                nc.gpsimd.dma_start(
                    out=xt[64 : 64 + FDIM, :],
                    in_=feats_p[t0, :, p * GP : (p + 1) * GP],
                )
                return xt

            def mk_tile(pool, shape, dt, tag):
                tmp = pool.tile(shape, dt, tag=tag)
                return tmp

            XPRE = 6  # steps of feature prefetch
            for rnd in range(npass // NL):
                lanes = [rnd * NL + v for v in range(NL)]
                xq = [[dma_x(v, p, t0) for t0 in range(min(XPRE, nsteps))]
                      for v, p in enumerate(lanes)]
                h2prev = [None] * NL
                for t in range(nsteps):
                    P = [None] * NL
                    h = [None] * NL
                    h2 = [None] * NL
                    d_new = [None] * NL
                    x = [None] * NL
                    for v, p in enumerate(lanes):
                        if t + XPRE < nsteps:
                            xq[v].append(dma_x(v, p, t + XPRE))
                        x[v] = xq[v][t]
                        P[v] = mk_tile(ppool, [128, (G // 2) * CH], F32, f"pp{v}")
                        h[v] = mk_tile(spool, [128, (G // 2) * CH], BF, f"h{v}")
                        h2[v] = mk_tile(spool, [128, (G // 2) * CH], BF, f"h2{v}")
                        d_new[v] = mk_tile(spool, [49, CH], BF, f"d{v}")

                    # chunk parity -> rhs partition half; vlane parity flips
                    # the output half so rotation covers all 4 quadrants
                    def pH(c, v):   # pre1 / h partitions
                        return 64 * ((c % 2) ^ (v % 2))

                    def pH2(c):  # pre2 / h2 partitions (and x row copy)
                        return 64 * (c % 2)

                    # Wave-skewed emission: vlane pairs (0,1) and (2,3)
                    # advance as two staggered waves so each wave's
                    # activations overlap the other wave's matmuls on the
                    # in-order engine queues. Within a wave, the (v, par)
                    # order rotates all 4 disjoint 64x64 quadrant sets.
                    bcol = (3, 4) if t == 0 else (0, 1)
                    dblk = DBANK * CH

                    def L1(vs):
                        for pair in range(G // 2):
                            for v in vs:
                                for par in range(2):
                                    c = 2 * pair + par
                                    blk = pair * CH
                                    o = P[v][pH(c, v) : pH(c, v) + HID, blk : blk + CH]
                                    nc.tensor.matmul(
                                        o,
                                        wp[pH2(c) : pH2(c) + FDIM, W1A_OFF : W1A_OFF + HID],
                                        x[v][pH2(c) : pH2(c) + FDIM, c * CH : (c + 1) * CH],
                                        start=True,
                                        stop=(t == 0),
                                        tile_position=(pH2(c), pH(c, v)),
                                    )
                        if t > 0:
                            for pair in range(G // 2):
                                for v in vs:
                                    for par in range(2):
                                        c = 2 * pair + par
                                        blk = pair * CH
                                        o = P[v][pH(c, v) : pH(c, v) + HID, blk : blk + CH]
                                        nc.tensor.matmul(
                                            o,
                                            wp[pH2(c) : pH2(c) + HID, M_OFF : M_OFF + HID],
                                            h2prev[v][pH2(c) : pH2(c) + HID, blk : blk + CH],
                                            start=False,
                                            stop=True,
                                            tile_position=(pH2(c), pH(c, v)),
                                        )

                    def ACT1(v):
                        # both halves in parallel: one on ACT, one on DVE
                        # (side alternates with vlane parity for balance)
                        for hf in range(2):
                            sl = slice(hf * CH, (hf + 1) * CH)
                            if hf == v % 2:
                                nc.scalar.activation(
                                    h[v][:, sl], P[v][:, sl],
                                    mybir.ActivationFunctionType.Relu,
                                    bias=bp[:, bcol[0] : bcol[1]], scale=1.0,
                                )
                            else:
                                nc.vector.tensor_scalar(
                                    h[v][:, sl], P[v][:, sl],
                                    bp[:, bcol[0] : bcol[1]], 0.0,
                                    mybir.AluOpType.add, mybir.AluOpType.max,
                                )

                    def L2(vs):
                        for pair in range(G // 2):
                            for v in vs:
                                for par in range(2):
                                    c = 2 * pair + par
                                    blk = pair * CH
                                    nc.tensor.matmul(
                                        P[v][pH2(c) : pH2(c) + HID, blk : blk + CH],
                                        wp[pH(c, v) : pH(c, v) + HID, W2_OFF : W2_OFF + HID],
                                        h[v][pH(c, v) : pH(c, v) + HID, blk : blk + CH],
                                        start=True,
                                        stop=True,
                                        tile_position=(pH(c, v), pH2(c)),
                                    )

                    def ACT2(v):
                        # both halves in parallel: one on ACT, one on DVE
                        for hf in range(2):
                            sl = slice(hf * CH, (hf + 1) * CH)
                            if hf != v % 2:
                                nc.scalar.activation(
                                    h2[v][:, sl], P[v][:, sl],
                                    mybir.ActivationFunctionType.Relu,
                                    bias=bp[:, 1:2], scale=1.0,
                                )
                            else:
                                nc.vector.tensor_scalar(
                                    h2[v][:, sl], P[v][:, sl],
                                    bp[:, 1:2], 0.0,
                                    mybir.AluOpType.add, mybir.AluOpType.max,
                                )
                        h2prev[v] = h2[v]

                    def L3(vs):
                        # independent single-shot pair-matmuls: pair i ->
                        # own 32-row region 32*i (no accumulation-group
                        # serialization); shared select block puts chunk
                        # 2i+par at row 32*i + 16*par
                        for v in vs:
                            for i in range(G // 2):
                                reg = 32 * i
                                nc.tensor.matmul(
                                    P[v][reg : reg + 32, dblk : dblk + CH],
                                    wp[0:128, W3_OFF : W3_OFF + 32],
                                    h2[v][0:128, i * CH : (i + 1) * CH],
                                    start=True,
                                    stop=True,
                                    tile_position=(0, reg),
                                )

                    def ACT3(v):
                        # chunk c=2i+par at row 32*i+16*par; stride-32
                        # partition APs keep the DMA count at 2 per vlane
                        p = lanes[v]
                        if v % 2 == 0:
                            nc.scalar.activation(
                                d_new[v][0:49, :], P[v][0:49, dblk : dblk + CH],
                                mybir.ActivationFunctionType.Copy,
                                bias=float(b3val), scale=1.0,
                            )
                        else:
                            nc.vector.tensor_scalar(
                                d_new[v][0:49, :], P[v][0:49, dblk : dblk + CH],
                                float(b3val), None, mybir.AluOpType.add,
                            )
                        for par in range(2):
                            nc.sync.dma_start(
                                out=out_p[t, par, p * (G // 2) : (p + 1) * (G // 2), :],
                                in_=d_new[v][16 * par : 16 * par + 33 : 32, :],
                            )

                    L1([0, 1])
                    ACT1(0)
                    ACT1(1)
                    L1([2, 3])
                    ACT1(2)
                    ACT1(3)
                    L2([0, 1])
                    ACT2(0)
                    ACT2(1)
                    L2([2, 3])
                    ACT2(2)
                    ACT2(3)
                    L3([0, 1])
                    ACT3(0)
                    ACT3(1)
                    L3([2, 3])
                    ACT3(2)
                    ACT3(3)
    return nc


LAST_RESULT = None


def kernel(**inputs):
    return _run(inputs, NPASS, T)


def _prepare(inputs, npass, nsteps):
    features = np.asarray(inputs["features"], dtype=np.float32)
    W1 = np.asarray(inputs["W1"], dtype=np.float32)
    b1 = np.asarray(inputs["b1"], dtype=np.float32)
    W2 = np.asarray(inputs["W2"], dtype=np.float32)
    b2 = np.asarray(inputs["b2"], dtype=np.float32)
    W3 = np.asarray(inputs["W3"], dtype=np.float32)
    b3 = np.asarray(inputs["b3"], dtype=np.float32)

    nc = _build_graph(npass, nsteps, float(b3[0]))
    nc.finalize()

    # host-side packing
    bf = ml_dtypes.bfloat16
    wpack = np.zeros((128, WCOLS), np.float32)
    M = W3 @ W1[3:4]  # (64, 64) rank-1: M[i, j] = W3[i] * W1[3, j]
    for half in (0, 64):
        wpack[half : half + 3, W1A_OFF : W1A_OFF + HID] = W1[0:3]
        wpack[half : half + HID, M_OFF : M_OFF + HID] = M
        wpack[half : half + HID, W2_OFF : W2_OFF + HID] = W2
    wpack[0:HID, W3_OFF + 0] = W3[:, 0]
    wpack[64 : 64 + HID, W3_OFF + 16] = W3[:, 0]
    wpack = wpack.astype(bf)

    b1p = b1 + W1[3] * b3[0]
    biasp = np.zeros((128, 4), np.float32)
    for half in (0, 64):
        biasp[half : half + HID, 0] = b1p
        biasp[half : half + HID, 1] = b2
        biasp[half : half + HID, 3] = b1
    biasp[:, 2] = b3[0]

    in_maps = []
    for k in range(NCORES):
        sh = features[k * NC : (k + 1) * NC]          # (NC, T, 3)
        feats = np.ascontiguousarray(sh.transpose(1, 2, 0)).astype(bf)  # (T,3,NC)
        in_maps.append({"feats": feats, "wpack": wpack, "biasp": biasp})

    return nc, in_maps


def _run(inputs, npass, nsteps, trace=False):
    global LAST_RESULT
    from concourse.bass_utils import run_bass_kernel_spmd

    nc, in_maps = _prepare(inputs, npass, nsteps)
    res = run_bass_kernel_spmd(
        nc, in_maps, core_ids=list(range(NCORES)), trace=trace
    )
    LAST_RESULT = res
    outs = res.results

    full = np.empty((N_TOT, T), np.float32)
    for k in range(NCORES):
        o = np.asarray(outs[k]["out"]).astype(np.float32)  # (T, 2, 32, CH)
        # o[t, par, p*(G//2) + j, :] holds chunk c=2j+par of pass p
        o = o.reshape(T, 2, NPASS, G // 2, CH).transpose(0, 2, 3, 1, 4)
        full[k * NC : (k + 1) * NC, :] = o.reshape(T, NC).T
    return full


if __name__ == "__main__":
    import reference

    inputs = reference.setup_inputs()
    out = kernel(**{k: np.asarray(v) for k, v in inputs.items()})
    print("kernel out", out.shape, out.dtype)
